# revision 1
# baseline (speedup 1.0000x reference)
"""Self-contained Trainium2 Bass kernel for nn_BestEllipseLoss_5720896438361.

kernel(output, target): full [512,128,128] f32 inputs -> scalar f32 loss.
Shards batch across 8 NeuronCores (64 samples each), one SPMD Bass kernel.
"""
import sys
if "/opt/trn_rl_repo" not in sys.path:
    sys.path.insert(0, "/opt/trn_rl_repo")

import numpy as np

import concourse.bass as bass
import concourse.bacc as bacc
import concourse.tile as tile
import concourse.mybir as mybir
import concourse.bass_isa as bass_isa

F32 = mybir.dt.float32
BF16 = mybir.dt.bfloat16
I32 = mybir.dt.int32
I16 = mybir.dt.int16
Alu = mybir.AluOpType
Act = mybir.ActivationFunctionType
AX = mybir.AxisListType

EPS = np.float32(1e-8)
LEVELS = [np.float32(0.3), np.float32(0.4), np.float32(0.5), np.float32(0.6), np.float32(0.7)]
NL = 5
H = 128
W = 128
NPIX = float(H * W)

_x = np.arange(W, dtype=np.float64)
_y = np.arange(H, dtype=np.float64)
# basis order j: {1, y, y^2, x, x*y, x^2}
C_B = np.array([
    H * W, W * _y.sum(), W * (_y ** 2).sum(),
    H * _x.sum(), _x.sum() * _y.sum(), H * (_x ** 2).sum(),
], dtype=np.float64).astype(np.float32)


def emit(nc, tc, NS=64, debug=False):
    F = NS * W
    SL = NL * NS          # l-major (l*NS + s)
    HSL = SL // 2         # gather round size

    dbg = {}
    def DBG(name, ap):
        if not debug:
            return
        t = nc.dram_tensor(f"dbg_{name}", list(ap.shape), ap.dtype, kind="ExternalOutput")
        nc.sync.dma_start(t[...], ap)
        dbg[name] = t

    t_in = nc.dram_tensor("t", [NS, H, W], F32, kind="ExternalInput")
    o_in = nc.dram_tensor("o", [NS, H, W], F32, kind="ExternalInput")
    loss_out = nc.dram_tensor("loss", [NS, 1], F32, kind="ExternalOutput")

    with tc.tile_pool(name="big", bufs=1) as big, \
         tc.tile_pool(name="med", bufs=1) as med, \
         tc.tile_pool(name="sml", bufs=1) as sml, \
         tc.tile_pool(name="fld", bufs=1) as fld, \
         tc.tile_pool(name="ps", bufs=2, space="PSUM") as ps, \
         tc.tile_pool(name="ps1", bufs=1, space="PSUM") as ps1:

        # ================= constants =================
        yi = sml.tile([128, 1], I32)
        nc.gpsimd.iota(yi[:], pattern=[[0, 1]], base=0, channel_multiplier=1)
        yv = sml.tile([128, 1], F32)
        nc.vector.tensor_copy(yv[:], yi[:])
        y2v = sml.tile([128, 1], F32)
        nc.vector.tensor_tensor(out=y2v[:], in0=yv[:], in1=yv[:], op=Alu.mult)

        ei = med.tile([128, 128], I32, tag="scrA")
        nc.gpsimd.iota(ei[:], pattern=[[1, 128]], base=0, channel_multiplier=-1)
        eif = med.tile([128, 128], F32, tag="scrB")
        nc.vector.tensor_copy(eif[:], ei[:])
        eye128 = med.tile([128, 128], F32)
        nc.vector.tensor_scalar(eye128[:], eif[:], 0.0, None, Alu.is_equal)

        e16i = sml.tile([128, 16], I32)
        nc.gpsimd.iota(e16i[:], pattern=[[1, 16]], base=0, channel_multiplier=-1)
        e16s = sml.tile([128, 16], F32)
        nc.vector.tensor_copy(e16s[:], e16i[:])
        nc.vector.tensor_scalar(e16s[:], e16s[:], 1.0 / 16.0, None, Alu.mult)
        e16t = sml.tile([128, 16], I32)
        nc.vector.tensor_copy(e16t[:], e16s[:])
        e16tf = sml.tile([128, 16], F32)
        nc.vector.tensor_copy(e16tf[:], e16t[:])
        eye16 = sml.tile([128, 16], F32)
        nc.vector.tensor_tensor(out=eye16[:], in0=e16s[:], in1=e16tf[:], op=Alu.is_equal)
        eyeneg16 = sml.tile([128, 16], F32)
        nc.vector.tensor_scalar(eyeneg16[:], eye16[:], -1.0, None, Alu.mult)

        onescol = sml.tile([128, 1], F32)
        nc.gpsimd.memset(onescol[:], 1.0)

        lvl_bias = []
        for l in range(NL):
            b = sml.tile([128, 1], F32, name=f"lvlb{l}")
            nc.gpsimd.memset(b[:], -float(LEVELS[l] - np.float32(0.5)))
            lvl_bias.append(b)

        # SEL_t [48, 6] fold matrices: SEL_t[q, m] = (q == t*12+m) or (q == t*12+6+m)
        di = med.tile([48, 6], I32, tag="scrC")
        nc.gpsimd.iota(di[:], pattern=[[-1, 6]], base=0, channel_multiplier=1)
        df = med.tile([48, 6], F32, tag="scrD")
        nc.vector.tensor_copy(df[:], di[:])
        SELS = []
        for t in range(4):
            s1 = med.tile([48, 6], F32, name=f"sel{t}")
            nc.vector.tensor_scalar(s1[:], df[:], float(12 * t), None, Alu.is_equal)
            s2 = med.tile([48, 6], F32, name=f"sel2_{t}", tag="scrE")
            nc.vector.tensor_scalar(s2[:], df[:], float(12 * t + 6), None, Alu.is_equal)
            nc.vector.tensor_tensor(out=s1[:], in0=s1[:], in1=s2[:], op=Alu.add)
            SELS.append(s1)

        # ---- moment lhsT table [128, 32*48] bf16 ----
        mast = med.tile([128, 768], F32, tag="scrA")
        nc.gpsimd.memset(mast[:], 1.0)
        xri = sml.tile([1, W], I32)
        nc.gpsimd.iota(xri[:], pattern=[[1, W]], base=0, channel_multiplier=0)
        xrf = sml.tile([1, W], F32)
        nc.vector.tensor_copy(xrf[:], xri[:])
        x2rf = sml.tile([1, W], F32)
        nc.vector.tensor_tensor(out=x2rf[:], in0=xrf[:], in1=xrf[:], op=Alu.mult)
        xfull = med.tile([128, W], F32, tag="scrB")
        nc.gpsimd.partition_broadcast(xfull[:], xrf[:], channels=128)
        x2full = med.tile([128, W], F32, tag="scrF")
        nc.gpsimd.partition_broadcast(x2full[:], x2rf[:], channels=128)
        mv = mast[:].rearrange("p (g t j) -> p g t j", g=32, t=4)
        xfv = xfull[:].rearrange("p (g t) -> p g t", g=32)
        x2fv = x2full[:].rearrange("p (g t) -> p g t", g=32)
        nc.vector.tensor_copy(mv[:, :, :, 3:4], xfv.to_broadcast((128, 32, 4, 1)))
        nc.vector.tensor_copy(mv[:, :, :, 4:5], xfv.to_broadcast((128, 32, 4, 1)))
        nc.vector.tensor_copy(mv[:, :, :, 5:6], x2fv.to_broadcast((128, 32, 4, 1)))
        mgt = mast[:].rearrange("p (gt j) -> p gt j", j=6)
        nc.vector.tensor_scalar(mgt[:, :, 1:2], mgt[:, :, 1:2], yv[:], None, Alu.mult)
        nc.vector.tensor_scalar(mgt[:, :, 4:5], mgt[:, :, 4:5], yv[:], None, Alu.mult)
        nc.vector.tensor_scalar(mgt[:, :, 2:3], mgt[:, :, 2:3], y2v[:], None, Alu.mult)
        hi24 = med.tile([128, 768], BF16, tag="scrG")
        nc.vector.tensor_copy(hi24[:], mast[:])
        hi24f = med.tile([128, 768], F32, tag="bcM0")
        nc.vector.tensor_copy(hi24f[:], hi24[:])
        lo24 = med.tile([128, 768], F32, tag="bcK1")
        nc.vector.tensor_tensor(out=lo24[:], in0=mast[:], in1=hi24f[:], op=Alu.subtract)
        table = med.tile([128, 32 * 48], BF16)
        tvv = table[:].rearrange("p (g t j) -> p g t j", g=32, t=4)
        nc.vector.tensor_copy(tvv[:, :, :, 0:6], hi24[:].rearrange("p (g t j) -> p g t j", g=32, t=4))
        nc.vector.tensor_copy(tvv[:, :, :, 6:12], lo24[:].rearrange("p (g t j) -> p g t j", g=32, t=4))

        # sbase [128, SL]: value at (l, s) = W*s
        sbi = med.tile([128, SL], I32, tag="tq")
        nc.gpsimd.iota(sbi[:].rearrange("p (l s) -> p l s", l=NL),
                       pattern=[[0, NL], [W, NS]], base=0, channel_multiplier=0)
        sbase = med.tile([128, SL], F32)
        nc.vector.tensor_copy(sbase[:], sbi[:])

        # lvl rows [NS, NL]
        lr_i = sml.tile([NS, NL], I32)
        nc.gpsimd.iota(lr_i[:], pattern=[[1, NL]], base=0, channel_multiplier=0)
        lvl_row = sml.tile([NS, NL], F32)
        nc.vector.tensor_copy(lvl_row[:], lr_i[:])
        nc.vector.tensor_scalar(lvl_row[:], lvl_row[:], 0.1, 0.3, Alu.mult, Alu.add)
        lvlp_row = sml.tile([NS, NL], F32)
        nc.vector.tensor_scalar(lvlp_row[:], lvl_row[:], -0.5, None, Alu.add)

        # ================= loads =================
        vt = big.tile([128, F], F32, tag="bigA")
        nc.sync.dma_start(vt[:].rearrange("p (s x) -> p s x", s=NS),
                          t_in[:, :, :].rearrange("s y x -> y s x"))
        vo = big.tile([128, F], F32, tag="bigO")
        nc.sync.dma_start(vo[:].rearrange("p (s x) -> p s x", s=NS),
                          o_in[:, :, :].rearrange("s y x -> y s x"))

        # ================= min/max =================
        def minmax_bcast(v, sfx):
            mxp = med.tile([128, NS], F32, name=f"mxp{sfx}", tag="mxp")
            nc.vector.tensor_reduce(mxp[:], v[:].rearrange("p (s x) -> p s x", s=NS), AX.X, Alu.max)
            mnp = med.tile([128, NS], F32, name=f"mnp{sfx}", tag="mnp")
            nc.vector.tensor_reduce(mnp[:], v[:].rearrange("p (s x) -> p s x", s=NS), AX.X, Alu.min)
            mxb = med.tile([128, NS], F32, name=f"mxb{sfx}")
            nc.gpsimd.partition_all_reduce(mxb[:], mxp[:], channels=128, reduce_op=bass_isa.ReduceOp.max)
            nc.vector.tensor_scalar(mnp[:], mnp[:], -1.0, None, Alu.mult)
            mnb = med.tile([128, NS], F32, name=f"mnb{sfx}")
            nc.gpsimd.partition_all_reduce(mnb[:], mnp[:], channels=128, reduce_op=bass_isa.ReduceOp.max)
            nc.vector.tensor_scalar(mnb[:], mnb[:], -1.0, None, Alu.mult)
            rngb = med.tile([128, NS], F32, name=f"rngb{sfx}")
            nc.vector.tensor_tensor(out=rngb[:], in0=mxb[:], in1=mnb[:], op=Alu.subtract)
            return mnb, rngb

        mnbT, rngbT = minmax_bcast(vt, "T")
        rngpT = med.tile([128, NS], F32)
        nc.vector.tensor_scalar(rngpT[:], rngbT[:], float(EPS), None, Alu.add)
        rbT = med.tile([128, NS], F32)
        nc.vector.reciprocal(rbT[:], rngpT[:])
        shiftT = med.tile([128, NS], F32)
        nc.vector.tensor_scalar(shiftT[:], rngpT[:], 0.5, None, Alu.mult)
        nc.vector.tensor_tensor(out=shiftT[:], in0=shiftT[:], in1=mnbT[:], op=Alu.add)

        # u' = (v - shift) * r, in place on vt
        nc.vector.tensor_tensor(out=vt[:].rearrange("p (s x) -> p s x", s=NS),
                                in0=vt[:].rearrange("p (s x) -> p s x", s=NS),
                                in1=shiftT[:].to_broadcast((128, NS, W)), op=Alu.subtract)
        nc.vector.tensor_tensor(out=vt[:].rearrange("p (s x) -> p s x", s=NS),
                                in0=vt[:].rearrange("p (s x) -> p s x", s=NS),
                                in1=rbT[:].to_broadcast((128, NS, W)), op=Alu.mult)
        up = vt

        # ================= prefix scan =================
        P = big.tile([128, F + 1], F32, tag="bigP")
        nc.gpsimd.memset(P[:, 0:1], 0.0)
        nc.vector.tensor_tensor_scan(P[:, 1:F + 1], up[:], up[:], 0.0, Alu.add, Alu.bypass)

        # ================= fields + moments =================
        # SA: sample-major collected moments: cols fl*6+j ; fl = l (F-moms), 5+l (G-moms)
        SA = med.tile([NS, 72], F32)

        def moments(field_tile, fl):
            psm = ps.tile([48, NS * 4], F32, name=f"psm{fl}", tag="psmom")
            fv = field_tile[:].rearrange("p (s x) -> p s x", s=NS)
            for g in range(32):
                nc.tensor.matmul(psm[:], table[:, g * 48:(g + 1) * 48],
                                 fv[:, :, 4 * g:4 * g + 4],
                                 start=(g == 0), stop=(g == 31))
            S48 = med.tile([48, NS * 4], F32, name=f"s48_{fl}", tag="s48")
            nc.vector.tensor_copy(S48[:], psm[:])
            pT = ps.tile([NS, 6], F32, name=f"pT{fl}", tag="ps6")
            sv = S48[:].rearrange("q (s t) -> q s t", s=NS)
            for t in range(4):
                nc.tensor.matmul(pT[:], sv[:, :, t], SELS[t][:],
                                 start=(t == 0), stop=(t == 3))
            nc.vector.tensor_copy(SA[:, fl * 6:fl * 6 + 6], pT[:])

        for l in range(NL):
            lvlp = float(LEVELS[l] - np.float32(0.5))
            f_l = fld.tile([128, F], BF16, name=f"f{l}", tag="ffield")
            nc.vector.tensor_scalar(f_l[:], up[:], lvlp, lvlp, Alu.max, Alu.subtract)
            g_l = fld.tile([128, F], BF16, name=f"g{l}", tag="gfield")
            nc.scalar.activation(g_l[:], up[:], Act.Sign, bias=lvl_bias[l][:], scale=1.0)
            moments(f_l, l)
            moments(g_l, 5 + l)

        # ================= i_tot =================
        btile = med.tile([128, NS + 1], F32)
        nc.vector.tensor_copy(btile[:], P[:, 0:F + 1:W])
        psb = ps1.tile([1, NS + 1], F32, tag="psmisc")
        nc.tensor.matmul(psb[:], onescol[:], btile[:], start=True, stop=True)
        brow = sml.tile([1, NS + 1], F32)
        nc.vector.tensor_copy(brow[:], psb[:])
        itot = sml.tile([1, NS], F32)
        nc.vector.tensor_tensor(out=itot[:], in0=brow[:, 1:NS + 1], in1=brow[:, 0:NS], op=Alu.subtract)
        nc.vector.tensor_scalar(itot[:], itot[:], 0.5 * NPIX, float(EPS), Alu.add, Alu.add)
        itr = sml.tile([1, NS], F32)
        nc.vector.reciprocal(itr[:], itot[:])

        # ================= target params (sample layout [NS, NL]) =================
        def FA(j):
            return SA[:, j:j + 30:6]

        def GA(j):
            return SA[:, 30 + j:30 + j + 30:6]

        def new5(nm):
            return med.tile([NS, NL], F32, name=nm)

        Wm = {}
        for j in range(6):
            cb = float(C_B[j])
            mk = med.tile([NS, NL], F32, name=f"mk{j}")
            nc.vector.tensor_scalar(mk[:], GA(j), cb, 0.5, Alu.add, Alu.mult)
            Wj = new5(f"W{j}")
            nc.vector.tensor_tensor(out=Wj[:], in0=mk[:], in1=lvl_row[:], op=Alu.mult)
            nc.vector.tensor_tensor(out=Wj[:], in0=Wj[:], in1=FA(j), op=Alu.add)
            Wm[j] = Wj

        def fit_params(Wd, tagp, n_l, want_roots):
            def nt(nm):
                return med.tile([NS, n_l], F32, name=tagp + nm)
            m00_ = nt("m00")
            nc.vector.tensor_scalar(m00_[:], Wd[0][:], float(EPS), None, Alu.add)
            im_ = nt("im")
            nc.vector.reciprocal(im_[:], m00_[:])
            cx_ = nt("cx"); cy_ = nt("cy"); tz = nt("tz")
            nc.vector.tensor_tensor(out=cx_[:], in0=Wd[3][:], in1=im_[:], op=Alu.mult)
            nc.vector.tensor_tensor(out=cy_[:], in0=Wd[1][:], in1=im_[:], op=Alu.mult)
            mu20_ = nt("mu20"); mu02_ = nt("mu02"); mu11_ = nt("mu11")
            nc.vector.tensor_tensor(out=mu20_[:], in0=Wd[5][:], in1=im_[:], op=Alu.mult)
            nc.vector.tensor_tensor(out=tz[:], in0=cx_[:], in1=cx_[:], op=Alu.mult)
            nc.vector.tensor_tensor(out=mu20_[:], in0=mu20_[:], in1=tz[:], op=Alu.subtract)
            nc.vector.tensor_tensor(out=mu02_[:], in0=Wd[2][:], in1=im_[:], op=Alu.mult)
            nc.vector.tensor_tensor(out=tz[:], in0=cy_[:], in1=cy_[:], op=Alu.mult)
            nc.vector.tensor_tensor(out=mu02_[:], in0=mu02_[:], in1=tz[:], op=Alu.subtract)
            nc.vector.tensor_tensor(out=mu11_[:], in0=Wd[4][:], in1=im_[:], op=Alu.mult)
            nc.vector.tensor_tensor(out=tz[:], in0=cx_[:], in1=cy_[:], op=Alu.mult)
            nc.vector.tensor_tensor(out=mu11_[:], in0=mu11_[:], in1=tz[:], op=Alu.subtract)
            dmu_ = nt("dmu"); smu_ = nt("smu"); cc_ = nt("cc")
            nc.vector.tensor_tensor(out=dmu_[:], in0=mu20_[:], in1=mu02_[:], op=Alu.subtract)
            nc.vector.tensor_tensor(out=smu_[:], in0=mu20_[:], in1=mu02_[:], op=Alu.add)
            nc.vector.tensor_tensor(out=cc_[:], in0=dmu_[:], in1=dmu_[:], op=Alu.mult)
            nc.vector.tensor_tensor(out=tz[:], in0=mu11_[:], in1=mu11_[:], op=Alu.mult)
            nc.vector.tensor_scalar(tz[:], tz[:], 4.0, None, Alu.mult)
            nc.vector.tensor_tensor(out=cc_[:], in0=cc_[:], in1=tz[:], op=Alu.add)
            com_ = nt("com")
            nc.scalar.sqrt(com_[:], cc_[:])
            gd = nt("gd"); rc = nt("rc")
            nc.vector.tensor_scalar(gd[:], com_[:], 1e-30, None, Alu.max)
            nc.vector.reciprocal(rc[:], gd[:])
            nc.vector.tensor_tensor(out=rc[:], in0=cc_[:], in1=rc[:], op=Alu.mult)
            nc.vector.tensor_tensor(out=com_[:], in0=com_[:], in1=rc[:], op=Alu.add)
            nc.vector.tensor_scalar(com_[:], com_[:], 0.5, None, Alu.mult)
            a2_ = nt("a2"); b2_ = nt("b2")
            nc.vector.tensor_tensor(out=a2_[:], in0=smu_[:], in1=com_[:], op=Alu.add)
            nc.vector.tensor_scalar(a2_[:], a2_[:], 2.0, float(EPS), Alu.mult, Alu.max)
            nc.vector.tensor_tensor(out=b2_[:], in0=smu_[:], in1=com_[:], op=Alu.subtract)
            nc.vector.tensor_scalar(b2_[:], b2_[:], 2.0, float(EPS), Alu.mult, Alu.max)
            a_ = nt("a"); b_ = nt("b")
            nc.scalar.sqrt(a_[:], a2_[:])
            nc.vector.tensor_scalar(gd[:], a_[:], 1e-30, None, Alu.max)
            nc.vector.reciprocal(rc[:], gd[:])
            nc.vector.tensor_tensor(out=rc[:], in0=a2_[:], in1=rc[:], op=Alu.mult)
            nc.vector.tensor_tensor(out=a_[:], in0=a_[:], in1=rc[:], op=Alu.add)
            nc.vector.tensor_scalar(a_[:], a_[:], 0.5, None, Alu.mult)
            nc.scalar.sqrt(b_[:], b2_[:])
            nc.vector.tensor_scalar(gd[:], b_[:], 1e-30, None, Alu.max)
            nc.vector.reciprocal(rc[:], gd[:])
            nc.vector.tensor_tensor(out=rc[:], in0=b2_[:], in1=rc[:], op=Alu.mult)
            nc.vector.tensor_tensor(out=b_[:], in0=b_[:], in1=rc[:], op=Alu.add)
            nc.vector.tensor_scalar(b_[:], b_[:], 0.5, None, Alu.mult)
            cg = nt("cg"); ic = nt("ic")
            nc.vector.tensor_scalar(cg[:], com_[:], 1e-30, None, Alu.max)
            nc.vector.reciprocal(ic[:], cg[:])
            cphi_ = nt("cphi"); sphi_ = nt("sphi")
            nc.vector.tensor_tensor(out=cphi_[:], in0=dmu_[:], in1=ic[:], op=Alu.mult)
            nc.vector.tensor_scalar(cphi_[:], cphi_[:], -1.0, 1.0, Alu.max, Alu.min)
            nc.vector.tensor_tensor(out=sphi_[:], in0=mu11_[:], in1=ic[:], op=Alu.mult)
            cth_ = nt("cth"); sth_ = nt("sth"); sg_ = nt("sg")
            nc.vector.tensor_scalar(cth_[:], cphi_[:], 1.0, 0.5, Alu.add, Alu.mult)
            nc.scalar.sqrt(cth_[:], cth_[:])
            nc.vector.tensor_scalar(sth_[:], cphi_[:], -1.0, None, Alu.mult)
            nc.vector.tensor_scalar(sth_[:], sth_[:], 1.0, 0.5, Alu.add, Alu.mult)
            nc.scalar.sqrt(sth_[:], sth_[:])
            nc.vector.tensor_scalar(sg_[:], sphi_[:], 0.0, None, Alu.is_ge)
            nc.vector.tensor_scalar(sg_[:], sg_[:], 2.0, -1.0, Alu.mult, Alu.add)
            nc.vector.tensor_tensor(out=sth_[:], in0=sth_[:], in1=sg_[:], op=Alu.mult)
            res = dict(cx=cx_, cy=cy_, cth=cth_, sth=sth_, a=a_, b=b_)
            if not want_roots:
                return res
            # roots coefs
            Aa = nt("Aa"); Bb = nt("Bb")
            nc.vector.tensor_scalar(Aa[:], a_[:], float(EPS), None, Alu.add)
            nc.vector.tensor_scalar(Bb[:], b_[:], float(EPS), None, Alu.add)
            iA2 = nt("iA2"); iB2 = nt("iB2")
            nc.vector.tensor_tensor(out=gd[:], in0=Aa[:], in1=Aa[:], op=Alu.mult)
            nc.vector.reciprocal(iA2[:], gd[:])
            nc.vector.tensor_tensor(out=gd[:], in0=Bb[:], in1=Bb[:], op=Alu.mult)
            nc.vector.reciprocal(iB2[:], gd[:])
            c2t = nt("c2t"); s2t = nt("s2t")
            nc.vector.tensor_tensor(out=c2t[:], in0=cth_[:], in1=cth_[:], op=Alu.mult)
            nc.vector.tensor_tensor(out=s2t[:], in0=sth_[:], in1=sth_[:], op=Alu.mult)
            Pq = nt("Pq")
            nc.vector.tensor_tensor(out=Pq[:], in0=c2t[:], in1=iA2[:], op=Alu.mult)
            nc.vector.tensor_tensor(out=tz[:], in0=s2t[:], in1=iB2[:], op=Alu.mult)
            nc.vector.tensor_tensor(out=Pq[:], in0=Pq[:], in1=tz[:], op=Alu.add)
            Rq = nt("Rq")
            nc.vector.tensor_tensor(out=Rq[:], in0=iA2[:], in1=iB2[:], op=Alu.subtract)
            nc.vector.tensor_tensor(out=Rq[:], in0=Rq[:], in1=cth_[:], op=Alu.mult)
            nc.vector.tensor_tensor(out=Rq[:], in0=Rq[:], in1=sth_[:], op=Alu.mult)
            K3 = nt("K3")
            nc.vector.tensor_tensor(out=K3[:], in0=iA2[:], in1=iB2[:], op=Alu.mult)
            iP = nt("iP")
            nc.vector.reciprocal(iP[:], Pq[:])
            K1 = nt("K1")
            nc.vector.tensor_tensor(out=K1[:], in0=Rq[:], in1=iP[:], op=Alu.mult)
            M0 = nt("M0")
            nc.vector.tensor_tensor(out=M0[:], in0=K1[:], in1=cy_[:], op=Alu.mult)
            nc.vector.tensor_tensor(out=M0[:], in0=M0[:], in1=cx_[:], op=Alu.add)
            H2 = nt("H2")
            nc.vector.tensor_scalar(H2[:], K3[:], -1.0, None, Alu.mult)
            H1 = nt("H1")
            nc.vector.tensor_tensor(out=H1[:], in0=K3[:], in1=cy_[:], op=Alu.mult)
            nc.vector.tensor_scalar(H1[:], H1[:], 2.0, None, Alu.mult)
            H0 = nt("H0")
            nc.vector.tensor_tensor(out=tz[:], in0=H1[:], in1=cy_[:], op=Alu.mult)
            nc.vector.tensor_scalar(tz[:], tz[:], 0.5, None, Alu.mult)
            nc.vector.tensor_tensor(out=H0[:], in0=Pq[:], in1=tz[:], op=Alu.subtract)
            res.update(M0=M0, K1=K1, H0=H0, H1=H1, H2=H2, iP=iP)
            return res

        pt = fit_params(Wm, "pt", NL, True)

        # ================= back-broadcast coef rows =================
        BS = med.tile([NS, 30], F32)
        for qi, q in enumerate([pt["M0"], pt["K1"], pt["H0"], pt["H1"], pt["H2"], pt["iP"]]):
            nc.vector.tensor_copy(BS[:, qi * 5:qi * 5 + 5], q[:])
        psb2 = ps1.tile([30, NS], F32, tag="psmisc")
        nc.tensor.transpose(psb2[:], BS[:, :], eye128[0:NS, 0:NS])
        BT = med.tile([30, NS], F32)
        nc.vector.tensor_copy(BT[:], psb2[:])
        bc = {}
        for qi, nm in enumerate(["M0", "K1", "H0", "H1", "H2", "iP"]):
            row = sml.tile([1, SL], F32, name=f"row{nm}")
            nc.sync.dma_start(row[:].rearrange("p (l s) -> p l s", l=NL),
                              BT[qi * 5:qi * 5 + 5, :])
            t128 = med.tile([128, SL], F32, name=f"bc{nm}", tag="bc" + nm)
            nc.gpsimd.partition_broadcast(t128[:], row[:], channels=128)
            bc[nm] = t128

        # ================= roots [128, SL] =================
        arg = med.tile([128, SL], F32)
        nc.vector.tensor_scalar(arg[:], bc["H2"][:], y2v[:], None, Alu.mult)
        tq = med.tile([128, SL], F32, tag="tq")
        nc.vector.tensor_scalar(tq[:], bc["H1"][:], yv[:], None, Alu.mult)
        nc.vector.tensor_tensor(out=arg[:], in0=arg[:], in1=tq[:], op=Alu.add)
        nc.vector.tensor_tensor(out=arg[:], in0=arg[:], in1=bc["H0"][:], op=Alu.add)
        valid = med.tile([128, SL], F32)
        nc.vector.tensor_scalar(valid[:], arg[:], 0.0, None, Alu.is_ge)
        nc.vector.tensor_scalar(arg[:], arg[:], 0.0, None, Alu.max)
        rt = med.tile([128, SL], F32)
        nc.scalar.sqrt(rt[:], arg[:])
        rrec = med.tile([128, SL], F32, tag="tq2")
        nc.vector.tensor_scalar(rrec[:], rt[:], 1e-30, None, Alu.max)
        nc.vector.reciprocal(rrec[:], rrec[:])
        nc.vector.tensor_tensor(out=rrec[:], in0=arg[:], in1=rrec[:], op=Alu.mult)
        nc.vector.tensor_tensor(out=rt[:], in0=rt[:], in1=rrec[:], op=Alu.add)
        nc.vector.tensor_scalar(rt[:], rt[:], 0.5, None, Alu.mult)
        half = rt  # in place: half = rt * iP
        nc.vector.tensor_tensor(out=half[:], in0=rt[:], in1=bc["iP"][:], op=Alu.mult)
        mrow = med.tile([128, SL], F32)
        nc.vector.tensor_scalar(mrow[:], bc["K1"][:], yv[:], None, Alu.mult)
        nc.vector.tensor_tensor(out=mrow[:], in0=bc["M0"][:], in1=mrow[:], op=Alu.subtract)
        xlo = med.tile([128, SL], F32)
        nc.vector.tensor_tensor(out=xlo[:], in0=mrow[:], in1=half[:], op=Alu.subtract)
        nc.vector.tensor_scalar(xlo[:], xlo[:], 0.0, 127.0, Alu.max, Alu.min)
        xhi = med.tile([128, SL], F32)
        nc.vector.tensor_tensor(out=xhi[:], in0=mrow[:], in1=half[:], op=Alu.add)
        nc.vector.tensor_scalar(xhi[:], xhi[:], 0.0, 127.0, Alu.max, Alu.min)
        nint = med.tile([128, SL], I32, tag="nint")
        nc.vector.tensor_copy(nint[:], xhi[:])
        nhi = med.tile([128, SL], F32)
        nc.vector.tensor_copy(nhi[:], nint[:])
        fhi = med.tile([128, SL], F32, tag="tq3")
        nc.vector.tensor_tensor(out=fhi[:], in0=nhi[:], in1=xhi[:], op=Alu.is_gt)
        nc.vector.tensor_tensor(out=nhi[:], in0=nhi[:], in1=fhi[:], op=Alu.subtract)
        nc.vector.tensor_copy(nint[:], xlo[:])
        nlo = med.tile([128, SL], F32)
        nc.vector.tensor_copy(nlo[:], nint[:])
        frac = med.tile([128, SL], F32, tag="tq3")
        nc.vector.tensor_tensor(out=frac[:], in0=xlo[:], in1=nlo[:], op=Alu.is_gt)
        nc.vector.tensor_tensor(out=nlo[:], in0=nlo[:], in1=frac[:], op=Alu.add)
        cnt = med.tile([128, SL], F32)
        nc.vector.tensor_tensor(out=cnt[:], in0=nhi[:], in1=nlo[:], op=Alu.subtract)
        nc.vector.tensor_scalar(cnt[:], cnt[:], 1.0, 0.0, Alu.add, Alu.max)
        nc.vector.tensor_tensor(out=cnt[:], in0=cnt[:], in1=valid[:], op=Alu.mult)
        # idx tiles (int16)
        nc.vector.tensor_scalar(nhi[:], nhi[:], 1.0, None, Alu.add)
        nc.vector.tensor_tensor(out=nhi[:], in0=nhi[:], in1=valid[:], op=Alu.mult)
        nc.vector.tensor_tensor(out=nhi[:], in0=nhi[:], in1=sbase[:], op=Alu.add)
        nc.vector.tensor_tensor(out=nlo[:], in0=nlo[:], in1=valid[:], op=Alu.mult)
        nc.vector.tensor_tensor(out=nlo[:], in0=nlo[:], in1=sbase[:], op=Alu.add)
        ih16 = med.tile([128, SL], I16, tag="ih16")
        nc.vector.tensor_copy(ih16[:], nhi[:])
        il16 = med.tile([128, SL], I16, tag="il16")
        nc.vector.tensor_copy(il16[:], nlo[:])

        # ================= gathers + I1/I0 =================
        psI1 = ps1.tile([1, SL], F32, tag="psI1")
        for r in range(2):
            Ghi = big.tile([128, HSL * 16], F32, name=f"ghi{r}", tag="bigA")
            nc.gpsimd.ap_gather(Ghi[:], P[:], ih16[:, r * HSL:(r + 1) * HSL],
                                channels=128, num_elems=F + 1, d=1, num_idxs=HSL * 16)
            Glo = big.tile([128, HSL * 16], F32, name=f"glo{r}", tag="bigL")
            nc.gpsimd.ap_gather(Glo[:], P[:], il16[:, r * HSL:(r + 1) * HSL],
                                channels=128, num_elems=F + 1, d=1, num_idxs=HSL * 16)
            gvh = Ghi[:].rearrange("p (sl j) -> p sl j", j=16)
            gvl = Glo[:].rearrange("p (sl j) -> p sl j", j=16)
            for j in range(16):
                nc.tensor.matmul(psI1[:, r * HSL:(r + 1) * HSL], eye16[:, j:j + 1],
                                 gvh[:, :, j], start=(j == 0), stop=False)
            for j in range(16):
                nc.tensor.matmul(psI1[:, r * HSL:(r + 1) * HSL], eyeneg16[:, j:j + 1],
                                 gvl[:, :, j], start=False, stop=(j == 15))
        psI0 = ps1.tile([1, SL], F32, tag="psI0")
        nc.tensor.matmul(psI0[:], onescol[:], cnt[:], start=True, stop=True)

        # ================= metric + argmax =================
        I1r = sml.tile([1, SL], F32)
        nc.vector.tensor_copy(I1r[:], psI1[:])
        I0r = sml.tile([1, SL], F32)
        nc.vector.tensor_copy(I0r[:], psI0[:])
        iin = sml.tile([1, SL], F32)
        nc.vector.tensor_scalar(iin[:], I0r[:], 0.5, None, Alu.mult)
        nc.vector.tensor_tensor(out=iin[:], in0=iin[:], in1=I1r[:], op=Alu.add)
        met = sml.tile([1, SL], F32)
        nc.vector.tensor_tensor(out=met[:].rearrange("p (l s) -> p s l", l=NL),
                                in0=iin[:].rearrange("p (l s) -> p s l", l=NL),
                                in1=itr[:].to_broadcast((1, NS, NL)), op=Alu.mult)
        nc.vector.tensor_scalar(I0r[:], I0r[:], float(1.0 / NPIX), None, Alu.mult)
        nc.vector.tensor_tensor(out=met[:], in0=met[:], in1=I0r[:], op=Alu.subtract)
        mmax = sml.tile([1, NS], F32)
        nc.vector.tensor_reduce(mmax[:], met[:].rearrange("p (l s) -> p s l", l=NL), AX.X, Alu.max)
        lidx_i = sml.tile([1, SL], I32)
        nc.gpsimd.iota(lidx_i[:].rearrange("p (l s) -> p l s", l=NL),
                       pattern=[[1, NL], [0, NS]], base=0, channel_multiplier=0)
        cand = sml.tile([1, SL], F32)
        nc.vector.tensor_copy(cand[:], lidx_i[:])
        eqmax = sml.tile([1, SL], F32)
        nc.vector.tensor_tensor(out=eqmax[:].rearrange("p (l s) -> p s l", l=NL),
                                in0=met[:].rearrange("p (l s) -> p s l", l=NL),
                                in1=mmax[:].to_broadcast((1, NS, NL)), op=Alu.is_lt)
        # cand = l + 99*(met < max)
        nc.vector.tensor_scalar(eqmax[:], eqmax[:], 99.0, None, Alu.mult)
        nc.vector.tensor_tensor(out=cand[:], in0=cand[:], in1=eqmax[:], op=Alu.add)
        bestr = sml.tile([1, NS], F32)
        nc.vector.tensor_reduce(bestr[:], cand[:].rearrange("p (l s) -> p s l", l=NL), AX.X, Alu.min)

        # ================= output side =================
        mnbO, rngbO = minmax_bcast(vo, "O")
        lvlfr = sml.tile([1, NS], F32)
        nc.vector.tensor_scalar(lvlfr[:], bestr[:], 0.1, 0.3, Alu.mult, Alu.add)
        lvlfb = med.tile([128, NS], F32)
        nc.gpsimd.partition_broadcast(lvlfb[:], lvlfr[:], channels=128)
        taub = med.tile([128, NS], F32)
        nc.vector.tensor_tensor(out=taub[:], in0=lvlfb[:], in1=rngbO[:], op=Alu.mult)
        nc.vector.tensor_tensor(out=taub[:], in0=taub[:], in1=mnbO[:], op=Alu.add)

        fo = fld.tile([128, F], BF16, tag="ffield")
        nc.vector.scalar_tensor_tensor(fo[:].rearrange("p (s x) -> p s x", s=NS),
                                       vo[:].rearrange("p (s x) -> p s x", s=NS),
                                       0.0,
                                       taub[:].to_broadcast((128, NS, W)),
                                       Alu.add, Alu.subtract)
        nc.vector.tensor_scalar(fo[:], fo[:], 0.0, None, Alu.max)
        go = fld.tile([128, F], BF16, tag="gfield")
        nc.vector.scalar_tensor_tensor(go[:].rearrange("p (s x) -> p s x", s=NS),
                                       vo[:].rearrange("p (s x) -> p s x", s=NS),
                                       0.0,
                                       taub[:].to_broadcast((128, NS, W)),
                                       Alu.add, Alu.is_ge)
        SB = med.tile([NS, 12], F32)

        def momentsO(field_tile, col0):
            psm = ps.tile([48, NS * 4], F32, name=f"psmO{col0}", tag="psmom")
            fv = field_tile[:].rearrange("p (s x) -> p s x", s=NS)
            for g in range(32):
                nc.tensor.matmul(psm[:], table[:, g * 48:(g + 1) * 48],
                                 fv[:, :, 4 * g:4 * g + 4],
                                 start=(g == 0), stop=(g == 31))
            S48 = med.tile([48, NS * 4], F32, name=f"s48O{col0}", tag="s48")
            nc.vector.tensor_copy(S48[:], psm[:])
            pT = ps.tile([NS, 6], F32, name=f"pTO{col0}", tag="ps6")
            sv = S48[:].rearrange("q (s t) -> q s t", s=NS)
            for t in range(4):
                nc.tensor.matmul(pT[:], sv[:, :, t], SELS[t][:],
                                 start=(t == 0), stop=(t == 3))
            nc.vector.tensor_copy(SB[:, col0:col0 + 6], pT[:])

        momentsO(fo, 0)
        momentsO(go, 6)

        # transpose best/mnO/rngO rows to sample cols
        def row2col(rowap, nm):
            pr = ps1.tile([NS, 1], F32, name=f"pr{nm}", tag="psmisc")
            nc.tensor.transpose(pr[:], rowap, eye128[0:1, 0:1])
            c = med.tile([NS, 1], F32, name=f"col{nm}")
            nc.vector.tensor_copy(c[:], pr[:])
            return c

        bestc = row2col(bestr[:], "best")
        mnoc = row2col(mnbO[0:1, :], "mno")
        rngoc = row2col(rngbO[0:1, :], "rngo")

        def col(nm):
            return med.tile([NS, 1], F32, name=nm)

        lvfc = col("lvfc")
        nc.vector.tensor_scalar(lvfc[:], bestc[:], 0.1, 0.3, Alu.mult, Alu.add)
        tauc = col("tauc")
        nc.vector.tensor_tensor(out=tauc[:], in0=lvfc[:], in1=rngoc[:], op=Alu.mult)
        nc.vector.tensor_tensor(out=tauc[:], in0=tauc[:], in1=mnoc[:], op=Alu.add)
        tmn = col("tmn")
        nc.vector.tensor_tensor(out=tmn[:], in0=tauc[:], in1=mnoc[:], op=Alu.subtract)
        WmO = {}
        for j in range(6):
            cb = float(C_B[j])
            wj = col(f"WO{j}")
            nc.vector.tensor_tensor(out=wj[:], in0=SB[:, 6 + j:7 + j], in1=tmn[:], op=Alu.mult)
            nc.vector.tensor_tensor(out=wj[:], in0=wj[:], in1=SB[:, j:j + 1], op=Alu.add)
            WmO[j] = wj

        po = fit_params(WmO, "po", 1, False)

        # select target params at best level
        eqm = med.tile([NS, NL], F32, name="eqm")
        l5f = med.tile([NS, NL], F32, name="l5f")
        nc.vector.tensor_copy(l5f[:], lr_i[:])
        nc.vector.tensor_scalar(eqm[:], l5f[:], bestc[:], None, Alu.is_equal)

        def select(src, nm):
            o = med.tile([NS, 1], F32, name="sel" + nm)
            tmp = med.tile([NS, NL], F32, name="selt" + nm, tag="seltmp")
            nc.vector.tensor_tensor(out=tmp[:], in0=src[:], in1=eqm[:], op=Alu.mult)
            nc.vector.tensor_reduce(o[:], tmp[:], AX.X, Alu.add)
            return o

        cxT = select(pt["cx"], "cx"); cyT = select(pt["cy"], "cy")
        cthT = select(pt["cth"], "ct"); sthT = select(pt["sth"], "st")
        aT = select(pt["a"], "a"); bT = select(pt["b"], "b")

        # ================= sym loss =================
        sc = col("sc")
        nc.vector.tensor_tensor(out=sc[:], in0=po["a"][:], in1=po["b"][:], op=Alu.max)
        t1c = col("t1c")
        nc.vector.tensor_tensor(out=t1c[:], in0=aT[:], in1=bT[:], op=Alu.max)
        nc.vector.tensor_tensor(out=sc[:], in0=sc[:], in1=t1c[:], op=Alu.max)
        nc.vector.tensor_scalar(sc[:], sc[:], float(EPS), None, Alu.add)
        isc = col("isc")
        nc.vector.reciprocal(isc[:], sc[:])
        lossc = col("lossc")
        td = col("td")

        def sqdiff_acc(xo, xt, first=False):
            nc.vector.tensor_tensor(out=td[:], in0=xo, in1=xt, op=Alu.subtract)
            nc.vector.tensor_tensor(out=td[:], in0=td[:], in1=isc[:], op=Alu.mult)
            nc.vector.tensor_tensor(out=td[:], in0=td[:], in1=td[:], op=Alu.mult)
            if first:
                nc.vector.tensor_copy(lossc[:], td[:])
            else:
                nc.vector.tensor_tensor(out=lossc[:], in0=lossc[:], in1=td[:], op=Alu.add)

        sqdiff_acc(po["cx"][:], cxT[:], first=True)
        sqdiff_acc(po["cy"][:], cyT[:])
        sqdiff_acc(po["a"][:], aT[:])
        sqdiff_acc(po["b"][:], bT[:])
        nc.vector.tensor_scalar(lossc[:], lossc[:], 0.5, None, Alu.mult)
        csum = col("csum")
        nc.vector.tensor_tensor(out=csum[:], in0=po["cth"][:], in1=cthT[:], op=Alu.mult)
        nc.vector.tensor_tensor(out=td[:], in0=po["sth"][:], in1=sthT[:], op=Alu.mult)
        nc.vector.tensor_tensor(out=csum[:], in0=csum[:], in1=td[:], op=Alu.add)
        nc.vector.tensor_scalar(csum[:], csum[:], -1.0, 1.0, Alu.mult, Alu.add)
        nc.vector.tensor_tensor(out=lossc[:], in0=lossc[:], in1=csum[:], op=Alu.add)

        nc.sync.dma_start(loss_out[:, :], lossc[:])

        DBG("SA", SA[:, 0:60])
        DBG("met", met[:])
        DBG("bestr", bestr[:])
        DBG("itr", itr[:])
        DBG("I1r", I1r[:])
        DBG("I0cnt", cnt[0:128, :])
        DBG("SB", SB[:])
        DBG("ocx", po["cx"][:]); DBG("ocy", po["cy"][:])
        DBG("oa", po["a"][:]); DBG("ob", po["b"][:])
        DBG("octh", po["cth"][:]); DBG("osth", po["sth"][:])
        DBG("tcx", pt["cx"][:]); DBG("tcy", pt["cy"][:])
        DBG("ta", pt["a"][:]); DBG("tb", pt["b"][:])
        DBG("tcth", pt["cth"][:]); DBG("tsth", pt["sth"][:])
        DBG("xlo", xlo[:]); DBG("xhi", xhi[:])


def build(NS=64, num_devices=1, debug=False):
    nc = bacc.Bacc("TRN2", target_bir_lowering=False, debug=False, num_devices=num_devices)
    with tile.TileContext(nc) as tc:
        emit(nc, tc, NS=NS, debug=debug)
    nc.compile()
    return nc


# ======================================================================
# Host-side entry point: full inputs -> shard across 8 cores -> gather
# ======================================================================
_CACHED = {}


def _get_nc():
    if "nc" not in _CACHED:
        _CACHED["nc"] = build(NS=64, num_devices=8)
    return _CACHED["nc"]


def kernel(output, target):
    from concourse.bass_utils import run_bass_kernel_spmd

    output = np.ascontiguousarray(output, dtype=np.float32)
    target = np.ascontiguousarray(target, dtype=np.float32)
    B = output.shape[0]
    n_cores = 8
    per = B // n_cores
    nc = _get_nc()
    in_maps = []
    for c in range(n_cores):
        sl = slice(c * per, (c + 1) * per)
        in_maps.append({"t": target[sl], "o": output[sl]})
    res = run_bass_kernel_spmd(nc, in_maps, core_ids=list(range(n_cores)))
    losses = np.concatenate([r["loss"].reshape(-1) for r in res.results])
    return np.float32(losses.mean(dtype=np.float64))

